# revision 43
# baseline (speedup 1.0000x reference)
"""Multi-head attention (B=2, S=4096, HIDDEN=512, HEADS=8) on 8 TRN2 NeuronCores.

Sharding: 8 cores = 2 batches x 4 head-groups (2 heads each).
Core c handles batch b = c//4 and heads {2g, 2g+1} where g = c%4
(projection feature slice [g*128, (g+1)*128)).

Per-core kernel (single SPMD program, different input data per core):
  - Q^T/K^T/V^T projections from pre-transposed x^T (host supplies x[b].T),
    V^T then PE-transposed into natural [t, d] layout
  - scores computed transposed: S^T[t, s] = sum_d K^T[d,t] Q^T[d,s]
  - P^T = exp(S^T / 8) on ScalarE straight out of PSUM, two banks per op
  - PV matmul with a ones-column appended to V so the softmax denominator
    l[s] drops out of the same accumulation (row 64 of the ctx PSUM tile)
  - normalization: r = 1/l broadcast across partitions via a K=1 matmul
  - output projection vs Wo^T rows of this core's heads -> partial [S, 512]
Host sums the 4 partials per batch and adds bo.

Large matmuls run in float32r (fast fp32 mode, full PE rate at N>=512).
float32r is a real packed format: every fp32r operand is produced by a
compute instruction writing a float32r tile (DMA'd fp32 data is converted
once on VectorE).
"""

import sys

import numpy as np

B, S, HID, HEADS, HD = 2, 4096, 512, 8, 64
FSL = 128          # features per core = 2 heads * 64
NCORES = 8
QC = 512           # query-chunk width
NTB = S // 128     # 32 key blocks
NQC = S // QC      # 8 query chunks

_PROGRAM = None


def _ensure_imports():
    try:
        import concourse  # noqa: F401
    except ImportError:
        sys.path.insert(0, "/opt/trn_rl_repo")


def _build_program():
    _ensure_imports()
    import concourse.bacc as bacc
    import concourse.mybir as mybir
    import concourse.tile as tile
    from concourse.masks import make_identity

    f32 = mybir.dt.float32
    f32r = mybir.dt.float32r
    AF = mybir.ActivationFunctionType

    nc = bacc.Bacc(
        "TRN2",
        target_bir_lowering=False,
        debug=False,
        enable_asserts=False,
        num_devices=NCORES,
    )

    xT = nc.dram_tensor("xT", [HID, S], f32, kind="ExternalInput").ap()
    wqT = nc.dram_tensor("wqT", [HID, FSL], f32, kind="ExternalInput").ap()
    wkT = nc.dram_tensor("wkT", [HID, FSL], f32, kind="ExternalInput").ap()
    wvT = nc.dram_tensor("wvT", [HID, FSL], f32, kind="ExternalInput").ap()
    woT = nc.dram_tensor("woT", [FSL, HID], f32, kind="ExternalInput").ap()
    bq = nc.dram_tensor("bq", [FSL, 1], f32, kind="ExternalInput").ap()
    bk = nc.dram_tensor("bk", [FSL, 1], f32, kind="ExternalInput").ap()
    bv = nc.dram_tensor("bv", [FSL, 1], f32, kind="ExternalInput").ap()
    out = nc.dram_tensor("out", [S, HID], f32, kind="ExternalOutput").ap()

    with tile.TileContext(nc) as tc:
        with (
            tc.tile_pool(name="persist", bufs=1) as pp,
            tc.tile_pool(name="vp_pool", bufs=NTB) as vpp,
        ):
            kt = pp.tile([FSL, S], f32r, tag="kt")
            qt = pp.tile([FSL, S], f32r, tag="qt")
            ctx01 = pp.tile([FSL, S], f32r, tag="ctx01")
            wo_r = pp.tile([FSL, HID], f32r, tag="wo_r")
            ones = pp.tile([128, 128], f32, tag="ones")
            ident = pp.tile([128, 128], f32r, tag="ident")
            bq_sb = pp.tile([FSL, 1], f32, tag="bq_sb")
            bk_sb = pp.tile([FSL, 1], f32, tag="bk_sb")
            bv_sb = pp.tile([FSL, 1], f32, tag="bv_sb")

            # biases ride the SWDGE ring so they don't occupy the HWDGE
            # rings' heads (each HWDGE descriptor costs ~0.6us regardless
            # of size)
            nc.gpsimd.dma_start(bq_sb[:], bq[:])
            nc.gpsimd.dma_start(bk_sb[:], bk[:])
            nc.gpsimd.dma_start(bv_sb[:], bv[:])
            nc.vector.memset(ones[:], 1.0)

            # warm the ACT exp table during the initial DMA window
            warm = pp.tile([128, 1], f32, tag="warm")
            nc.scalar.activation(warm[:], ones[:, 0:1], AF.Exp)

            vp_tiles = []

            with (
                tc.tile_pool(name="w_pool", bufs=1) as wp,
                tc.tile_pool(name="psB", bufs=1, space="PSUM") as psB,
                tc.tile_pool(name="pt_pool", bufs=9) as ptp,
                tc.tile_pool(name="small", bufs=3) as sp,
                tc.tile_pool(name="out_pool", bufs=4) as obp,
            ):
                # ---------------- projections ----------------
                ident_f32 = wp.tile([128, 128], f32, tag="ident_f32")
                make_identity(nc, ident_f32[:])
                nc.vector.tensor_copy(ident[:], ident_f32[:])

                # weights first: tiny DMAs must not queue behind the x chunks
                # (k/q before v; wo last -- it is not needed until the first
                # output projection)
                wqs, wks, wvs = [], [], []
                for name, dst, src in (("wk", wks, wkT), ("wq", wqs, wqT),
                                       ("wv", wvs, wvT)):
                    # one DMA + one convert per weight matrix: [512,128] DRAM
                    # folded to [128, 4, 128] (partition p, chunk i)
                    w_raw = wp.tile([128, 4, FSL], f32, tag="wraw", bufs=2)
                    nc.sync.dma_start(w_raw[:],
                                      src.rearrange("(i p) f -> p i f", p=128))
                    w_r = wp.tile([128, 4, FSL], f32r, tag=f"{name}r")
                    nc.scalar.copy(w_r[:], w_raw[:])
                    for i in range(4):
                        dst.append(w_r[:, i, :])

                # per-t8 pipeline: load x chunks (both HWDGE rings), convert
                # to rotating fp32r chunks, project K^T/Q^T/V^T, transpose V
                def emit_attn_pair(qc, h, tb2, ctx_pss):
                    hh = h * HD
                    st = psB.tile([128, 2, QC], f32, tag="st", bufs=2,
                                  name="st")
                    for j in range(2):
                        tb = tb2 * 2 + j
                        nc.tensor.matmul(
                            st[:, j, :],
                            kt[hh:hh + HD, tb * 128:(tb + 1) * 128],
                            qt[hh:hh + HD, qc * QC:(qc + 1) * QC],
                            start=True, stop=True)
                    pt = ptp.tile([128, 2, QC], f32r, tag="pt", name="pt")
                    nc.scalar.activation(pt[:], st[:], AF.Exp,
                                         scale=float(HD) ** -0.5)
                    for j in range(2):
                        tb = tb2 * 2 + j
                        nc.tensor.matmul(
                            ctx_pss[h][:],
                            vp_tiles[tb][:, h * (HD + 1):(h + 1) * (HD + 1)],
                            pt[:, j, :],
                            start=(tb == 0), stop=(tb == NTB - 1))

                def emit_attn_tail(qc, ctx_pss):
                    for h in range(2):
                        ctx_ps = ctx_pss[h]
                        # normalize: r = 1/l lives on partition HD (=64);
                        # reciprocal moves it to partition 0 (1-partition op,
                        # quadrant-aligned src -- the HW-verified case), then
                        # GPSIMD broadcasts it across the 64 ctx partitions
                        r_t = sp.tile([128, QC], f32, tag="r", name="r_t")
                        nc.vector.reciprocal(r_t[0:1, :],
                                             ctx_ps[HD:HD + 1, :])
                        rb_sb = sp.tile([HD, QC], f32, tag="rbs", name="rb_sb")
                        nc.gpsimd.partition_broadcast(rb_sb[:], r_t[0:1, :])
                        qs = slice(qc * QC, (qc + 1) * QC)
                        if h == 0:
                            nc.vector.tensor_mul(
                                ctx01[0:HD, qs], ctx_ps[0:HD, :], rb_sb[:])
                        else:
                            # partition-shift to rows 64-127 via two
                            # HW-verified 32-partition quadrant copies
                            tmp1 = sp.tile([HD, QC], f32r, tag="tmp1",
                                           name="tmp1")
                            nc.vector.tensor_mul(tmp1[:], ctx_ps[0:HD, :],
                                                 rb_sb[:])
                            nc.vector.tensor_copy(ctx01[HD:HD + 32, qs],
                                                  tmp1[0:32, :])
                            nc.vector.tensor_copy(ctx01[HD + 32:FSL, qs],
                                                  tmp1[32:HD, :])
                    for sc in range(QC // 128):
                        col = qc * QC + sc * 128
                        po = psB.tile([128, HID], f32, tag="misc", bufs=2,
                                      name="po")
                        nc.tensor.matmul(po[:], ctx01[:, col:col + 128],
                                         wo_r[:], start=True, stop=True)
                        ob = obp.tile([128, HID], f32, tag="ob", name="ob")
                        if qc == NQC - 1:
                            nc.scalar.copy(ob[:], po[:])
                        else:
                            nc.vector.tensor_copy(ob[:], po[:])
                        nc.sync.dma_start(out[col:col + 128, :], ob[:])

                ctx_q0 = [
                    psB.tile([HD + 1, QC], f32, tag="ctx", bufs=2,
                             name=f"ctxq0{h}")
                    for h in range(2)
                ]
                ctx_q1 = [
                    psB.tile([HD + 1, QC], f32, tag="ctx", bufs=2,
                             name=f"ctxq1{h}")
                    for h in range(2)
                ]
                Q1_EARLY = 4
                for t8 in range(NQC):
                    cs = slice(t8 * QC, (t8 + 1) * QC)
                    xcs = []
                    for i in range(4):
                        xt_raw = wp.tile([128, QC], f32, tag="xtraw", bufs=6)
                        eng = nc.sync if i % 2 == 0 else nc.scalar
                        eng.dma_start(xt_raw[:], xT[i * 128:(i + 1) * 128, cs])
                        xc = wp.tile([128, QC], f32r, tag="xc", bufs=8)
                        if t8 < 3:
                            nc.scalar.copy(xc[:], xt_raw[:])
                        else:
                            nc.vector.tensor_copy(xc[:], xt_raw[:])
                        xcs.append(xc)
                    for ws, bias_sb, dst in ((wks, bk_sb, kt), (wqs, bq_sb, qt)):
                        ps = psB.tile([FSL, QC], f32, tag="misc", bufs=2,
                                      name="kqps")
                        for i in range(4):
                            nc.tensor.matmul(
                                ps[:], ws[i][:], xcs[i][:],
                                start=(i == 0), stop=(i == 3))
                        nc.vector.tensor_scalar_add(dst[:, cs], ps[:], bias_sb[:])
                    psvT = psB.tile([FSL, QC], f32, tag="misc", bufs=2,
                                    name="vps")
                    for i in range(4):
                        nc.tensor.matmul(
                            psvT[:], wvs[i][:], xcs[i][:],
                            start=(i == 0), stop=(i == 3))
                    vt_sb = wp.tile([FSL, QC], f32r, tag="vt", bufs=2)
                    nc.vector.tensor_scalar_add(vt_sb[:], psvT[:], bv_sb[:])
                    for j in range(QC // 128):
                        vtp = psB.tile([128, FSL], f32r, tag="misc", bufs=2,
                                       name="vtp")
                        nc.tensor.transpose(vtp[:], vt_sb[:, j * 128:(j + 1) * 128],
                                            ident[:])
                        vp = vpp.tile([128, 2 * (HD + 1)], f32r, tag="vp")
                        nc.vector.tensor_copy(vp[:, 0:HD], vtp[:, 0:HD])
                        nc.vector.tensor_copy(vp[:, HD + 1:2 * HD + 1],
                                              vtp[:, HD:2 * HD])
                        nc.vector.tensor_copy(vp[:, HD:HD + 1], ones[:, 0:1])
                        nc.vector.tensor_copy(vp[:, 2 * HD + 1:2 * HD + 2],
                                              ones[:, 0:1])
                        vp_tiles.append(vp)
                    if t8 == 0:
                        # Wo load deferred off the scalar ring's head; it is
                        # not consumed until the first output projection
                        wo_raw = wp.tile([FSL, HID], f32, tag="wo_raw")
                        nc.scalar.dma_start(wo_raw[:], woT[:])
                        nc.vector.tensor_copy(wo_r[:], wo_raw[:])
                    # attention for qc=0 rides along as soon as this t8's
                    # K/Q/V chunks exist, keeping ScalarE fed during the
                    # projection phase
                    for tb2 in (2 * t8, 2 * t8 + 1):
                        for h in range(2):
                            emit_attn_pair(0, h, tb2, ctx_q0)
                    # late projection: qc=1's first score/exp groups ride
                    # along to fill ScalarE's idle windows (their PVs wait
                    # for ctx slots, the exps do not)
                    if t8 >= NQC - Q1_EARLY:
                        tb2e = t8 - (NQC - Q1_EARLY)
                        for h in range(2):
                            emit_attn_pair(1, h, tb2e, ctx_q1)
                emit_attn_tail(0, ctx_q0)

                # ---------------- attention + output projection ----------------
                for qc in range(1, NQC):
                    if qc == 1:
                        ctx_pss = ctx_q1
                        tb2_start = Q1_EARLY
                    else:
                        ctx_pss = [
                            psB.tile([HD + 1, QC], f32, tag="ctx", bufs=2,
                                     name=f"ctxps{h}")
                            for h in range(2)
                        ]
                        tb2_start = 0
                    for tb2 in range(tb2_start, NTB // 2):
                        for h in range(2):
                            emit_attn_pair(qc, h, tb2, ctx_pss)
                    emit_attn_tail(qc, ctx_pss)

    nc.compile()
    return nc


def _get_program():
    global _PROGRAM
    if _PROGRAM is None:
        _PROGRAM = _build_program()
    return _PROGRAM


def kernel(**inputs):
    _ensure_imports()
    from concourse import bass_utils

    x = np.ascontiguousarray(np.asarray(inputs["x"], dtype=np.float32))
    Wq = np.asarray(inputs["Wq"], dtype=np.float32)
    Wk = np.asarray(inputs["Wk"], dtype=np.float32)
    Wv = np.asarray(inputs["Wv"], dtype=np.float32)
    Wo = np.asarray(inputs["Wo"], dtype=np.float32)
    bq = np.asarray(inputs["bq"], dtype=np.float32)
    bk = np.asarray(inputs["bk"], dtype=np.float32)
    bv = np.asarray(inputs["bv"], dtype=np.float32)
    bo = np.asarray(inputs["bo"], dtype=np.float32)

    nc = _get_program()

    wqT_full = np.ascontiguousarray(Wq.T)
    wkT_full = np.ascontiguousarray(Wk.T)
    wvT_full = np.ascontiguousarray(Wv.T)
    woT_full = np.ascontiguousarray(Wo.T)

    in_maps = []
    for c in range(NCORES):
        b, g = divmod(c, 4)
        fs = slice(g * FSL, (g + 1) * FSL)
        in_maps.append({
            "xT": np.ascontiguousarray(x[b].T),
            "wqT": np.ascontiguousarray(wqT_full[:, fs]),
            "wkT": np.ascontiguousarray(wkT_full[:, fs]),
            "wvT": np.ascontiguousarray(wvT_full[:, fs]),
            "woT": np.ascontiguousarray(woT_full[fs, :]),
            "bq": np.ascontiguousarray(bq[fs].reshape(FSL, 1)),
            "bk": np.ascontiguousarray(bk[fs].reshape(FSL, 1)),
            "bv": np.ascontiguousarray(bv[fs].reshape(FSL, 1)),
        })

    res = bass_utils.run_bass_kernel_spmd(nc, in_maps,
                                          core_ids=list(range(NCORES)))
    outs = [r["out"] for r in res.results]

    full = np.empty((B, S, HID), dtype=np.float32)
    for b in range(B):
        full[b] = outs[4 * b] + outs[4 * b + 1] + outs[4 * b + 2] + outs[4 * b + 3]
        full[b] += bo
    return full
